# revision 17
# baseline (speedup 1.0000x reference)
"""Trainium2 Bass kernel for nn_AttentionLayer (B=2, S=2048, HID=1024, 16 heads x 64).

Sharding: 8 cores = 2 batches x 4 head-groups (4 heads each). Each core computes
its batch's attention for its 4 heads and writes a disjoint [256, 2048] slice of
the output (transposed). No collectives.

v4 design (all-bf16 datapath; ScalarE exp paces phase B, PE runs just under it):
  - inputs/weights stream in as bf16; DMA issue order staged [toT-h0, fromT-h0,
    toT-h1, fromT-h1] so the minimal head (K tb0-1, V tt0-3, Q fb0) unblocks
    the first exp ~20 us in.
  - Q/K projections: j-major [128, 512] matmul blocks, bias added on DVE.
    V projection: t-major [t, j] accumulation with a K=1 ones-row bias matmul
    and the softmax-denominator ones column appended (stationary for PV).
  - scores: S.T tiles [128t x 1024f] per head via row-packed K=64 matmuls;
    exp on ScalarE -> E bf16; PV matmuls trail the exp stream by one t-tile.
  - softmax epilogue entirely off the PE: DVE copy, DMA den row to partition 0,
    reciprocal_approx_fast, GpSimd partition-broadcast, DVE multiply, DMA out.
  - remaining projections are interleaved into the round streams as fillers;
    rounds ordered (jt0,fb0),(jt0,fb1),(jt1,fb0),(jt1,fb1) so K/Q for jt1 can
    be produced as fillers of the jt0 rounds.
"""
import numpy as np

B, S, HID = 2, 2048, 1024
NUM_HEADS, HEAD_DIM = 16, 64
G = 4                 # head-groups (cores per batch)
HPC = 4               # heads per core
JW = HPC * HEAD_DIM   # 256 W columns per core
NC_CHUNKS = HID // 128  # 8 contraction chunks
NT = S // 128         # 16 t tiles
NFB = 2               # f blocks of 1024
SCALE = 1.0 / np.sqrt(float(HEAD_DIM))

_cached = None


def _build():
    import contextlib
    import concourse.bass as bass
    import concourse.tile as tile
    from concourse import bacc, mybir

    F32 = mybir.dt.float32
    BF16 = mybir.dt.bfloat16
    Act = mybir.ActivationFunctionType

    nc = bacc.Bacc("TRN2", target_bir_lowering=False, debug=False, num_devices=8)

    fromT = nc.dram_tensor("fromT", (HID, S), BF16, kind="ExternalInput").ap()
    toT = nc.dram_tensor("toT", (HID, S), BF16, kind="ExternalInput").ap()
    wq = nc.dram_tensor("wq", (HID, JW), BF16, kind="ExternalInput").ap()
    wk = nc.dram_tensor("wk", (HID, JW), BF16, kind="ExternalInput").ap()
    wv = nc.dram_tensor("wv", (HID, JW), BF16, kind="ExternalInput").ap()
    bq = nc.dram_tensor("bq", (JW, 1), F32, kind="ExternalInput").ap()
    bk = nc.dram_tensor("bk", (JW, 1), F32, kind="ExternalInput").ap()
    bv = nc.dram_tensor("bv", (1, JW), BF16, kind="ExternalInput").ap()
    out = nc.dram_tensor("out", (JW, S), F32, kind="ExternalOutput").ap()

    with tile.TileContext(nc) as tc:
        with contextlib.ExitStack() as es:
            persist = es.enter_context(tc.tile_pool(name="persist", bufs=1))
            psbig = es.enter_context(tc.tile_pool(name="psbig", bufs=2, space="PSUM"))
            pssm = es.enter_context(tc.tile_pool(name="pssm", bufs=4, space="PSUM"))
            work = es.enter_context(tc.tile_pool(name="work", bufs=1))

            # --- K/V weights first on the DMA queue (needed by the first matmul)
            wk_sb = persist.tile([128, NC_CHUNKS, JW], BF16, tag="wk")
            nc.sync.dma_start(wk_sb[:], wk.rearrange("(c p) j -> p c j", p=128))
            wv_sb = persist.tile([128, NC_CHUNKS, JW], BF16, tag="wv")
            nc.sync.dma_start(wv_sb[:], wv.rearrange("(c p) j -> p c j", p=128))
            bv_row = persist.tile([1, JW], BF16, tag="bvrow")
            nc.sync.dma_start(bv_row[:], bv[0:1, :])
            ones_bf = persist.tile([1, 128], BF16, tag="onesbf")
            nc.vector.memset(ones_bf[:], 1.0)

            # --- persistent projection outputs (all bf16)
            qt = [persist.tile([128, S], BF16, tag=f"qt{jt}", name=f"qt{jt}") for jt in range(2)]
            kt = [persist.tile([128, S], BF16, tag=f"kt{jt}", name=f"kt{jt}") for jt in range(2)]
            vp = [persist.tile([128, HPC, 65], BF16, tag=f"vp{tt}", name=f"vp{tt}") for tt in range(NT)]

            # --- input streams: 4 column-halves of 1024, staged issue order
            SH = 1024
            xh = {}

            def load_half(nm, src, h, eng):
                # per-chunk entries so dependent proj matmuls stream chunk-wise
                x = work.tile([128, NC_CHUNKS, SH], BF16, tag="xh",
                              name=f"x_{nm}", bufs=4)
                xh[nm] = x
                for c in range(NC_CHUNKS):
                    eng.dma_start(
                        x[:, c, :],
                        src[128 * c:128 * c + 128, SH * h:SH * h + SH])

            # fromT rides the second HWDGE queue (ScalarE's) so it lands in
            # parallel with toT instead of behind it; ScalarE is idle pre-exp.
            load_half("t0", toT, 0, nc.sync)
            load_half("f0", fromT, 0, nc.scalar)
            # Q weights + biases behind the t0 stream (not needed until ~20us)
            wq_sb = persist.tile([128, NC_CHUNKS, JW], BF16, tag="wq")
            nc.sync.dma_start(wq_sb[:], wq.rearrange("(c p) j -> p c j", p=128))
            b_sb = {}
            for nm, src in (("bq", bq), ("bk", bk)):
                for jt in range(2):
                    t = persist.tile([128, 1], F32, tag=f"{nm}{jt}")
                    nc.sync.dma_start(t[:], src[128 * jt:128 * jt + 128, 0:1])
                    b_sb[(nm, jt)] = t

            # ---- helpers
            def proj_block(w_sb, bias_key, dst, jt, tb, xk):
                x = xh[xk]
                f2 = tb % 2
                acc = psbig.tile([128, 512], F32, tag="big",
                                 name=f"acc_{bias_key}{jt}_{tb}")
                for c in range(NC_CHUNKS):
                    nc.tensor.matmul(
                        acc[:],
                        w_sb[:, c, 128 * jt:128 * jt + 128],
                        x[:, c, 512 * f2:512 * f2 + 512],
                        start=(c == 0), stop=(c == NC_CHUNKS - 1))
                nc.vector.tensor_scalar_add(
                    dst[jt][:, 512 * tb:512 * tb + 512], acc[:],
                    b_sb[(bias_key, jt)][:])

            def k_proj(jt, tb):
                proj_block(wk_sb, "bk", kt, jt, tb, f"t{tb // 2}")

            def q_proj(jt, fbk):
                proj_block(wq_sb, "bq", qt, jt, fbk, f"f{fbk // 2}")

            def v_proj(tt):
                h, t2 = divmod(tt, NT // 2)
                tx = xh[f"t{h}"]
                accv = psbig.tile([128, 512], F32, tag="big", name=f"vacc{tt}")
                nc.tensor.matmul(accv[:, 0:JW], ones_bf[0:1, :], bv_row[0:1, :],
                                 start=True, stop=False)
                for c in range(NC_CHUNKS):
                    nc.tensor.matmul(
                        accv[:, 0:JW],
                        tx[:, c, 128 * t2:128 * t2 + 128],
                        wv_sb[:, c, :],
                        start=False, stop=(c == NC_CHUNKS - 1))
                nc.vector.memset(vp[tt][:, :, 64], 1.0)
                nc.vector.tensor_copy(
                    vp[tt][:, :, 0:64],
                    accv[:, 0:JW].rearrange("p (k e) -> p k e", k=HPC))

            # --- head: minimal work to unblock round 0
            v_proj(0)
            k_proj(0, 0)
            v_proj(1)
            k_proj(0, 1)
            v_proj(2)
            load_half("t1", toT, 1, nc.sync)
            q_proj(0, 0)
            q_proj(0, 1)
            v_proj(3)
            load_half("f1", fromT, 1, nc.scalar)

            # --- phase B rounds: (jt0,fb0), (jt0,fb1), (jt1,fb0), (jt1,fb1)
            rounds = [(0, 0), (0, 1), (1, 0), (1, 1)]  # (jt, fb)

            filler = {r: {tt: [] for tt in range(NT)} for r in range(4)}
            # V t-tiles 4..15 at round-0 tt 1..12 (tiles >=8 read toT-h1,
            # which lands on the DMA queue ~44us in — keep them late enough)
            for i in range(12):
                filler[0][1 + i].append(lambda tt=4 + i: v_proj(tt))
            filler[0][4].append(lambda: k_proj(0, 2))
            filler[0][6].append(lambda: k_proj(0, 3))
            filler[0][13].append(lambda: q_proj(0, 2))
            filler[0][14].append(lambda: q_proj(0, 3))
            # round 1 fillers: K jt1 + Q jt1 fb0 (needed by round 2)
            filler[1][0].append(lambda: k_proj(1, 0))
            filler[1][2].append(lambda: k_proj(1, 1))
            filler[1][4].append(lambda: k_proj(1, 2))
            filler[1][6].append(lambda: k_proj(1, 3))
            filler[1][9].append(lambda: q_proj(1, 0))
            filler[1][11].append(lambda: q_proj(1, 1))
            # round 2 fillers: Q jt1 fb1 (needed by round 3)
            filler[2][2].append(lambda: q_proj(1, 2))
            filler[2][5].append(lambda: q_proj(1, 3))

            def epilogue_half(r, dd, half, cacc, so):
                sbf = work.tile([65, 512], F32, tag="sbf", bufs=4,
                                name=f"sbf{r}_{dd}{half}")
                nc.vector.tensor_copy(sbf[:], cacc[:])
                # reciprocal_approx_fast and partition_broadcast both misread
                # sources at a nonzero partition offset on HW (sim is fine) —
                # DMA the denominator row down to partition 0 first.
                den0 = work.tile([1, 512], F32, tag="den0", bufs=4,
                                 name=f"den0{r}_{dd}{half}")
                nc.sync.dma_start(den0[:], sbf[64:65, :])
                rcp = work.tile([1, 512], F32, tag="rcp", bufs=4,
                                name=f"rcp{r}_{dd}{half}")
                nc.vector.reciprocal_approx_fast(rcp[:], den0[:])
                rcpb = work.tile([64, 512], F32, tag="rcpb", bufs=4,
                                 name=f"rcpb{r}_{dd}{half}")
                nc.gpsimd.partition_broadcast(rcpb[:], rcp[:])
                nc.vector.tensor_mul(so[:, 512 * half:512 * half + 512],
                                     sbf[0:64, :], rcpb[:])

            def epilogue_dd(r, jt, dd, cacc):
                # both halves share one [64, 1024] staging tile and one out DMA
                k_local = 2 * jt + dd
                fb = rounds[r][1]
                so = work.tile([64, 1024], F32, tag="so", bufs=2,
                               name=f"so{r}_{dd}")
                for half in range(2):
                    epilogue_half(r, dd, half, cacc[(dd, half)], so)
                nc.sync.dma_start(
                    out[64 * k_local:64 * k_local + 64,
                        1024 * fb:1024 * fb + 1024], so[:])

            for r, (jt, fb) in enumerate(rounds):
                fbase = 1024 * fb
                cacc = {}
                for dd in range(2):
                    for half in range(2):
                        cacc[(dd, half)] = pssm.tile(
                            [65, 512], F32, tag="sm", name=f"cacc{r}_{dd}{half}")
                E = {}

                def st_exp(tt):
                    for dd in range(2):
                        stp = psbig.tile([128, 1024], F32, tag="big",
                                         name=f"st{r}_{tt}_{dd}")
                        for half in range(2):
                            fo = fbase + 512 * half
                            nc.tensor.matmul(
                                stp[:, 512 * half:512 * half + 512],
                                kt[jt][64 * dd:64 * dd + 64, 128 * tt:128 * tt + 128],
                                qt[jt][64 * dd:64 * dd + 64, fo:fo + 512],
                                start=True, stop=True,
                                tile_position=(64 * dd, 0))
                        e = work.tile([128, 1024], BF16, tag="et", bufs=6,
                                      name=f"e{r}_{tt}_{dd}")
                        nc.scalar.activation(e[:], stp[:], Act.Exp,
                                             bias=0.0, scale=SCALE)
                        E[(dd, tt)] = e

                def pv(tt):
                    for dd in range(2):
                        k_local = 2 * jt + dd
                        for half in range(2):
                            nc.tensor.matmul(
                                cacc[(dd, half)][:],
                                vp[tt][:, k_local, :],
                                E[(dd, tt)][:, 512 * half:512 * half + 512],
                                start=(tt == 0), stop=(tt == NT - 1))

                for tt in range(NT):
                    st_exp(tt)
                    for thunk in filler[r][tt]:
                        thunk()
                    if tt > 0:
                        pv(tt - 1)
                pv(NT - 1)
                for dd in range(2):
                    epilogue_dd(r, jt, dd, cacc)

    nc.compile()
    return nc


def _get_nc():
    global _cached
    if _cached is None:
        _cached = _build()
    return _cached


def _numpy_fallback(from_tensor, to_tensor, attention_mask, Wq, bq, Wk, bk, Wv, bv):
    b, f, _ = from_tensor.shape
    t = to_tensor.shape[1]
    h, d = NUM_HEADS, HEAD_DIM
    q = (from_tensor @ Wq + bq).reshape(b, f, h, d).transpose(0, 2, 1, 3)
    k = (to_tensor @ Wk + bk).reshape(b, t, h, d).transpose(0, 2, 1, 3)
    v = (to_tensor @ Wv + bv).reshape(b, t, h, d).transpose(0, 2, 1, 3)
    scores = np.einsum("bhfd,bhtd->bhft", q, k) * (1.0 / np.sqrt(float(d)))
    adder = (1.0 - attention_mask[:, None].astype(np.float32)) * -10000.0
    scores = scores + adder
    scores = scores - scores.max(axis=-1, keepdims=True)
    e = np.exp(scores)
    probs = e / e.sum(axis=-1, keepdims=True)
    ctx = np.einsum("bhft,bhtd->bhfd", probs, v)
    return ctx.transpose(0, 2, 1, 3).reshape(b, f, h * d).astype(np.float32)


def _make_in_maps(from_tensor, to_tensor, Wq, bq, Wk, bk, Wv, bv):
    import ml_dtypes
    bf16 = ml_dtypes.bfloat16
    fromT = [np.ascontiguousarray(from_tensor[b].T).astype(bf16) for b in range(B)]
    toT = [np.ascontiguousarray(to_tensor[b].T).astype(bf16) for b in range(B)]
    in_maps = []
    for core in range(8):
        b, g = divmod(core, G)
        j0 = JW * g
        in_maps.append({
            "fromT": fromT[b],
            "toT": toT[b],
            "wq": np.ascontiguousarray(Wq[:, j0:j0 + JW]).astype(bf16),
            "wk": np.ascontiguousarray(Wk[:, j0:j0 + JW]).astype(bf16),
            "wv": np.ascontiguousarray(Wv[:, j0:j0 + JW]).astype(bf16),
            "bq": np.ascontiguousarray(bq[j0:j0 + JW].reshape(JW, 1)),
            "bk": np.ascontiguousarray(bk[j0:j0 + JW].reshape(JW, 1)),
            "bv": np.ascontiguousarray(bv[j0:j0 + JW].reshape(1, JW)).astype(bf16),
        })
    return in_maps


def profile_exec_time(inputs):
    """Rerun on HW with NTFF tracing; returns whole-NEFF exec time in ns."""
    from concourse import bass_utils
    nc = _get_nc()
    in_maps = _make_in_maps(
        np.asarray(inputs["from_tensor"], dtype=np.float32),
        np.asarray(inputs["to_tensor"], dtype=np.float32),
        np.asarray(inputs["Wq"], dtype=np.float32),
        np.asarray(inputs["bq"], dtype=np.float32),
        np.asarray(inputs["Wk"], dtype=np.float32),
        np.asarray(inputs["bk"], dtype=np.float32),
        np.asarray(inputs["Wv"], dtype=np.float32),
        np.asarray(inputs["bv"], dtype=np.float32))
    res = bass_utils.run_bass_kernel_spmd(nc, in_maps, core_ids=list(range(8)),
                                          trace=True)
    profile_exec_time.last_results = res
    return res.exec_time_ns


def kernel(**inputs) -> np.ndarray:
    from_tensor = np.asarray(inputs["from_tensor"], dtype=np.float32)
    to_tensor = np.asarray(inputs["to_tensor"], dtype=np.float32)
    attention_mask = np.asarray(inputs["attention_mask"])
    Wq = np.asarray(inputs["Wq"], dtype=np.float32)
    bq = np.asarray(inputs["bq"], dtype=np.float32)
    Wk = np.asarray(inputs["Wk"], dtype=np.float32)
    bk = np.asarray(inputs["bk"], dtype=np.float32)
    Wv = np.asarray(inputs["Wv"], dtype=np.float32)
    bv = np.asarray(inputs["bv"], dtype=np.float32)

    if not np.all(attention_mask == 1):
        # General-mask path (not exercised by the spec'd all-ones fill):
        # plain numpy reference math.
        return _numpy_fallback(from_tensor, to_tensor, attention_mask,
                               Wq, bq, Wk, bk, Wv, bv)

    from concourse import bass_utils

    nc = _get_nc()

    in_maps = _make_in_maps(from_tensor, to_tensor, Wq, bq, Wk, bk, Wv, bv)
    res = bass_utils.run_bass_kernel_spmd(nc, in_maps, core_ids=list(range(8)))
    kernel.last_results = res

    output = np.empty((B, S, HID), dtype=np.float32)
    for core in range(8):
        b, g = divmod(core, G)
        j0 = JW * g
        output[b, :, j0:j0 + JW] = res.results[core]["out"].T
    return output


if __name__ == "__main__":
    rng = np.random.default_rng(0)
    ins = {
        "from_tensor": rng.standard_normal((B, S, HID), dtype=np.float32),
        "to_tensor": rng.standard_normal((B, S, HID), dtype=np.float32),
        "attention_mask": np.ones((B, S, S), dtype=np.int32),
        "Wq": rng.standard_normal((HID, HID), dtype=np.float32) * 0.02,
        "bq": rng.standard_normal((HID,), dtype=np.float32) * 0.01,
        "Wk": rng.standard_normal((HID, HID), dtype=np.float32) * 0.02,
        "bk": rng.standard_normal((HID,), dtype=np.float32) * 0.01,
        "Wv": rng.standard_normal((HID, HID), dtype=np.float32) * 0.02,
        "bv": rng.standard_normal((HID,), dtype=np.float32) * 0.01,
    }
    got = kernel(**ins)
    want = _numpy_fallback(**ins)
    err = np.abs(got - want).max() / np.abs(want).max()
    print("self-test rel err:", err)


# revision 18
# speedup vs baseline: 1.0269x; 1.0269x over previous
"""Trainium2 Bass kernel for nn_AttentionLayer (B=2, S=2048, HID=1024, 16 heads x 64).

Sharding: 8 cores = 2 batches x 4 head-groups (4 heads each). Each core computes
its batch's attention for its 4 heads and writes a disjoint [256, 2048] slice of
the output (transposed). No collectives.

v4 design (all-bf16 datapath; ScalarE exp paces phase B, PE runs just under it):
  - inputs/weights stream in as bf16; DMA issue order staged [toT-h0, fromT-h0,
    toT-h1, fromT-h1] so the minimal head (K tb0-1, V tt0-3, Q fb0) unblocks
    the first exp ~20 us in.
  - Q/K projections: j-major [128, 512] matmul blocks, bias added on DVE.
    V projection: t-major [t, j] accumulation with a K=1 ones-row bias matmul
    and the softmax-denominator ones column appended (stationary for PV).
  - scores: S.T tiles [128t x 1024f] per head via row-packed K=64 matmuls;
    exp on ScalarE -> E bf16; PV matmuls trail the exp stream by one t-tile.
  - softmax epilogue entirely off the PE: DVE copy, DMA den row to partition 0,
    reciprocal_approx_fast, GpSimd partition-broadcast, DVE multiply, DMA out.
  - remaining projections are interleaved into the round streams as fillers;
    rounds ordered (jt0,fb0),(jt0,fb1),(jt1,fb0),(jt1,fb1) so K/Q for jt1 can
    be produced as fillers of the jt0 rounds.
"""
import numpy as np

B, S, HID = 2, 2048, 1024
NUM_HEADS, HEAD_DIM = 16, 64
G = 4                 # head-groups (cores per batch)
HPC = 4               # heads per core
JW = HPC * HEAD_DIM   # 256 W columns per core
NC_CHUNKS = HID // 128  # 8 contraction chunks
NT = S // 128         # 16 t tiles
NFB = 2               # f blocks of 1024
SCALE = 1.0 / np.sqrt(float(HEAD_DIM))

_cached = None


def _build():
    import contextlib
    import concourse.bass as bass
    import concourse.tile as tile
    from concourse import bacc, mybir

    F32 = mybir.dt.float32
    BF16 = mybir.dt.bfloat16
    Act = mybir.ActivationFunctionType

    nc = bacc.Bacc("TRN2", target_bir_lowering=False, debug=False, num_devices=8)

    fromT = nc.dram_tensor("fromT", (HID, S), BF16, kind="ExternalInput").ap()
    toT = nc.dram_tensor("toT", (HID, S), BF16, kind="ExternalInput").ap()
    wq = nc.dram_tensor("wq", (HID, JW), BF16, kind="ExternalInput").ap()
    wk = nc.dram_tensor("wk", (HID, JW), BF16, kind="ExternalInput").ap()
    wv = nc.dram_tensor("wv", (HID, JW), BF16, kind="ExternalInput").ap()
    bq = nc.dram_tensor("bq", (JW, 1), F32, kind="ExternalInput").ap()
    bk = nc.dram_tensor("bk", (JW, 1), F32, kind="ExternalInput").ap()
    bv = nc.dram_tensor("bv", (1, JW), BF16, kind="ExternalInput").ap()
    out = nc.dram_tensor("out", (JW, S), F32, kind="ExternalOutput").ap()

    with tile.TileContext(nc) as tc:
        with contextlib.ExitStack() as es:
            persist = es.enter_context(tc.tile_pool(name="persist", bufs=1))
            psbig = es.enter_context(tc.tile_pool(name="psbig", bufs=2, space="PSUM"))
            pssm = es.enter_context(tc.tile_pool(name="pssm", bufs=4, space="PSUM"))
            work = es.enter_context(tc.tile_pool(name="work", bufs=1))

            # --- weights first on the DMA queue (needed by the first matmul)
            wk_sb = persist.tile([128, NC_CHUNKS, JW], BF16, tag="wk")
            nc.sync.dma_start(wk_sb[:], wk.rearrange("(c p) j -> p c j", p=128))
            wv_sb = persist.tile([128, NC_CHUNKS, JW], BF16, tag="wv")
            nc.sync.dma_start(wv_sb[:], wv.rearrange("(c p) j -> p c j", p=128))
            b_sb = {}
            for nm, src in (("bq", bq), ("bk", bk)):
                for jt in range(2):
                    t = persist.tile([128, 1], F32, tag=f"{nm}{jt}")
                    nc.sync.dma_start(t[:], src[128 * jt:128 * jt + 128, 0:1])
                    b_sb[(nm, jt)] = t
            bv_row = persist.tile([1, JW], BF16, tag="bvrow")
            nc.sync.dma_start(bv_row[:], bv[0:1, :])
            ones_bf = persist.tile([1, 128], BF16, tag="onesbf")
            nc.vector.memset(ones_bf[:], 1.0)

            # --- persistent projection outputs (all bf16)
            qt = [persist.tile([128, S], BF16, tag=f"qt{jt}", name=f"qt{jt}") for jt in range(2)]
            kt = [persist.tile([128, S], BF16, tag=f"kt{jt}", name=f"kt{jt}") for jt in range(2)]
            vp = [persist.tile([128, HPC, 65], BF16, tag=f"vp{tt}", name=f"vp{tt}") for tt in range(NT)]

            # --- input streams: 4 column-halves of 1024, staged issue order
            SH = 1024
            xh = {}

            def load_half(nm, src, h):
                x = work.tile([128, NC_CHUNKS, SH], BF16, tag="xh",
                              name=f"x_{nm}", bufs=4)
                xh[nm] = x
                for c in range(NC_CHUNKS):
                    nc.sync.dma_start(
                        x[:, c, :],
                        src[128 * c:128 * c + 128, SH * h:SH * h + SH])

            load_half("t0", toT, 0)
            load_half("f0", fromT, 0)
            # Q weights behind the t0/f0 streams (not needed until ~25us)
            wq_sb = persist.tile([128, NC_CHUNKS, JW], BF16, tag="wq")
            nc.sync.dma_start(wq_sb[:], wq.rearrange("(c p) j -> p c j", p=128))

            # ---- helpers
            def proj_block(w_sb, bias_key, dst, jt, tb, xk):
                x = xh[xk]
                f2 = tb % 2
                acc = psbig.tile([128, 512], F32, tag="big",
                                 name=f"acc_{bias_key}{jt}_{tb}")
                for c in range(NC_CHUNKS):
                    nc.tensor.matmul(
                        acc[:],
                        w_sb[:, c, 128 * jt:128 * jt + 128],
                        x[:, c, 512 * f2:512 * f2 + 512],
                        start=(c == 0), stop=(c == NC_CHUNKS - 1))
                nc.vector.tensor_scalar_add(
                    dst[jt][:, 512 * tb:512 * tb + 512], acc[:],
                    b_sb[(bias_key, jt)][:])

            def k_proj(jt, tb):
                proj_block(wk_sb, "bk", kt, jt, tb, f"t{tb // 2}")

            def q_proj(jt, fbk):
                proj_block(wq_sb, "bq", qt, jt, fbk, f"f{fbk // 2}")

            def v_proj(tt):
                h, t2 = divmod(tt, NT // 2)
                tx = xh[f"t{h}"]
                accv = psbig.tile([128, 512], F32, tag="big", name=f"vacc{tt}")
                nc.tensor.matmul(accv[:, 0:JW], ones_bf[0:1, :], bv_row[0:1, :],
                                 start=True, stop=False)
                for c in range(NC_CHUNKS):
                    nc.tensor.matmul(
                        accv[:, 0:JW],
                        tx[:, c, 128 * t2:128 * t2 + 128],
                        wv_sb[:, c, :],
                        start=False, stop=(c == NC_CHUNKS - 1))
                nc.vector.memset(vp[tt][:, :, 64], 1.0)
                nc.vector.tensor_copy(
                    vp[tt][:, :, 0:64],
                    accv[:, 0:JW].rearrange("p (k e) -> p k e", k=HPC))

            # --- head: minimal work to unblock round 0
            v_proj(0)
            k_proj(0, 0)
            v_proj(1)
            k_proj(0, 1)
            v_proj(2)
            load_half("t1", toT, 1)
            q_proj(0, 0)
            q_proj(0, 1)
            v_proj(3)
            load_half("f1", fromT, 1)

            # --- phase B rounds: (jt0,fb0), (jt0,fb1), (jt1,fb0), (jt1,fb1)
            rounds = [(0, 0), (0, 1), (1, 0), (1, 1)]  # (jt, fb)

            filler = {r: {tt: [] for tt in range(NT)} for r in range(4)}
            # V t-tiles 4..15 at round-0 tt 1..12 (tiles >=8 read toT-h1,
            # which lands on the DMA queue ~44us in — keep them late enough)
            for i in range(12):
                filler[0][1 + i].append(lambda tt=4 + i: v_proj(tt))
            filler[0][4].append(lambda: k_proj(0, 2))
            filler[0][6].append(lambda: k_proj(0, 3))
            filler[0][13].append(lambda: q_proj(0, 2))
            filler[0][14].append(lambda: q_proj(0, 3))
            # round 1 fillers: K jt1 + Q jt1 fb0 (needed by round 2)
            filler[1][0].append(lambda: k_proj(1, 0))
            filler[1][2].append(lambda: k_proj(1, 1))
            filler[1][4].append(lambda: k_proj(1, 2))
            filler[1][6].append(lambda: k_proj(1, 3))
            filler[1][9].append(lambda: q_proj(1, 0))
            filler[1][11].append(lambda: q_proj(1, 1))
            # round 2 fillers: Q jt1 fb1 (needed by round 3)
            filler[2][2].append(lambda: q_proj(1, 2))
            filler[2][5].append(lambda: q_proj(1, 3))

            def epilogue_half(r, dd, half, cacc, so):
                sbf = work.tile([65, 512], F32, tag="sbf", bufs=4,
                                name=f"sbf{r}_{dd}{half}")
                nc.vector.tensor_copy(sbf[:], cacc[:])
                # reciprocal_approx_fast and partition_broadcast both misread
                # sources at a nonzero partition offset on HW (sim is fine) —
                # DMA the denominator row down to partition 0 first.
                den0 = work.tile([1, 512], F32, tag="den0", bufs=4,
                                 name=f"den0{r}_{dd}{half}")
                nc.sync.dma_start(den0[:], sbf[64:65, :])
                rcp = work.tile([1, 512], F32, tag="rcp", bufs=4,
                                name=f"rcp{r}_{dd}{half}")
                nc.vector.reciprocal_approx_fast(rcp[:], den0[:])
                rcpb = work.tile([64, 512], F32, tag="rcpb", bufs=4,
                                 name=f"rcpb{r}_{dd}{half}")
                nc.gpsimd.partition_broadcast(rcpb[:], rcp[:])
                nc.vector.tensor_mul(so[:, 512 * half:512 * half + 512],
                                     sbf[0:64, :], rcpb[:])

            def epilogue_dd(r, jt, dd, cacc):
                # both halves share one [64, 1024] staging tile and one out DMA
                k_local = 2 * jt + dd
                fb = rounds[r][1]
                so = work.tile([64, 1024], F32, tag="so", bufs=2,
                               name=f"so{r}_{dd}")
                for half in range(2):
                    epilogue_half(r, dd, half, cacc[(dd, half)], so)
                nc.sync.dma_start(
                    out[64 * k_local:64 * k_local + 64,
                        1024 * fb:1024 * fb + 1024], so[:])

            for r, (jt, fb) in enumerate(rounds):
                fbase = 1024 * fb
                cacc = {}
                for dd in range(2):
                    for half in range(2):
                        cacc[(dd, half)] = pssm.tile(
                            [65, 512], F32, tag="sm", name=f"cacc{r}_{dd}{half}")
                E = {}

                def st_exp(tt):
                    for dd in range(2):
                        stp = psbig.tile([128, 1024], F32, tag="big",
                                         name=f"st{r}_{tt}_{dd}")
                        for half in range(2):
                            fo = fbase + 512 * half
                            nc.tensor.matmul(
                                stp[:, 512 * half:512 * half + 512],
                                kt[jt][64 * dd:64 * dd + 64, 128 * tt:128 * tt + 128],
                                qt[jt][64 * dd:64 * dd + 64, fo:fo + 512],
                                start=True, stop=True,
                                tile_position=(64 * dd, 0))
                        e = work.tile([128, 1024], BF16, tag="et", bufs=6,
                                      name=f"e{r}_{tt}_{dd}")
                        nc.scalar.activation(e[:], stp[:], Act.Exp,
                                             bias=0.0, scale=SCALE)
                        E[(dd, tt)] = e

                def pv(tt):
                    for dd in range(2):
                        k_local = 2 * jt + dd
                        for half in range(2):
                            nc.tensor.matmul(
                                cacc[(dd, half)][:],
                                vp[tt][:, k_local, :],
                                E[(dd, tt)][:, 512 * half:512 * half + 512],
                                start=(tt == 0), stop=(tt == NT - 1))

                for tt in range(NT):
                    st_exp(tt)
                    for thunk in filler[r][tt]:
                        thunk()
                    if tt > 0:
                        pv(tt - 1)
                pv(NT - 1)
                for dd in range(2):
                    epilogue_dd(r, jt, dd, cacc)

    nc.compile()
    return nc


def _get_nc():
    global _cached
    if _cached is None:
        _cached = _build()
    return _cached


def _numpy_fallback(from_tensor, to_tensor, attention_mask, Wq, bq, Wk, bk, Wv, bv):
    b, f, _ = from_tensor.shape
    t = to_tensor.shape[1]
    h, d = NUM_HEADS, HEAD_DIM
    q = (from_tensor @ Wq + bq).reshape(b, f, h, d).transpose(0, 2, 1, 3)
    k = (to_tensor @ Wk + bk).reshape(b, t, h, d).transpose(0, 2, 1, 3)
    v = (to_tensor @ Wv + bv).reshape(b, t, h, d).transpose(0, 2, 1, 3)
    scores = np.einsum("bhfd,bhtd->bhft", q, k) * (1.0 / np.sqrt(float(d)))
    adder = (1.0 - attention_mask[:, None].astype(np.float32)) * -10000.0
    scores = scores + adder
    scores = scores - scores.max(axis=-1, keepdims=True)
    e = np.exp(scores)
    probs = e / e.sum(axis=-1, keepdims=True)
    ctx = np.einsum("bhft,bhtd->bhfd", probs, v)
    return ctx.transpose(0, 2, 1, 3).reshape(b, f, h * d).astype(np.float32)


def _make_in_maps(from_tensor, to_tensor, Wq, bq, Wk, bk, Wv, bv):
    import ml_dtypes
    bf16 = ml_dtypes.bfloat16
    fromT = [np.ascontiguousarray(from_tensor[b].T).astype(bf16) for b in range(B)]
    toT = [np.ascontiguousarray(to_tensor[b].T).astype(bf16) for b in range(B)]
    in_maps = []
    for core in range(8):
        b, g = divmod(core, G)
        j0 = JW * g
        in_maps.append({
            "fromT": fromT[b],
            "toT": toT[b],
            "wq": np.ascontiguousarray(Wq[:, j0:j0 + JW]).astype(bf16),
            "wk": np.ascontiguousarray(Wk[:, j0:j0 + JW]).astype(bf16),
            "wv": np.ascontiguousarray(Wv[:, j0:j0 + JW]).astype(bf16),
            "bq": np.ascontiguousarray(bq[j0:j0 + JW].reshape(JW, 1)),
            "bk": np.ascontiguousarray(bk[j0:j0 + JW].reshape(JW, 1)),
            "bv": np.ascontiguousarray(bv[j0:j0 + JW].reshape(1, JW)).astype(bf16),
        })
    return in_maps


def profile_exec_time(inputs):
    """Rerun on HW with NTFF tracing; returns whole-NEFF exec time in ns."""
    from concourse import bass_utils
    nc = _get_nc()
    in_maps = _make_in_maps(
        np.asarray(inputs["from_tensor"], dtype=np.float32),
        np.asarray(inputs["to_tensor"], dtype=np.float32),
        np.asarray(inputs["Wq"], dtype=np.float32),
        np.asarray(inputs["bq"], dtype=np.float32),
        np.asarray(inputs["Wk"], dtype=np.float32),
        np.asarray(inputs["bk"], dtype=np.float32),
        np.asarray(inputs["Wv"], dtype=np.float32),
        np.asarray(inputs["bv"], dtype=np.float32))
    res = bass_utils.run_bass_kernel_spmd(nc, in_maps, core_ids=list(range(8)),
                                          trace=True)
    profile_exec_time.last_results = res
    return res.exec_time_ns


def kernel(**inputs) -> np.ndarray:
    from_tensor = np.asarray(inputs["from_tensor"], dtype=np.float32)
    to_tensor = np.asarray(inputs["to_tensor"], dtype=np.float32)
    attention_mask = np.asarray(inputs["attention_mask"])
    Wq = np.asarray(inputs["Wq"], dtype=np.float32)
    bq = np.asarray(inputs["bq"], dtype=np.float32)
    Wk = np.asarray(inputs["Wk"], dtype=np.float32)
    bk = np.asarray(inputs["bk"], dtype=np.float32)
    Wv = np.asarray(inputs["Wv"], dtype=np.float32)
    bv = np.asarray(inputs["bv"], dtype=np.float32)

    if not np.all(attention_mask == 1):
        # General-mask path (not exercised by the spec'd all-ones fill):
        # plain numpy reference math.
        return _numpy_fallback(from_tensor, to_tensor, attention_mask,
                               Wq, bq, Wk, bk, Wv, bv)

    from concourse import bass_utils

    nc = _get_nc()

    in_maps = _make_in_maps(from_tensor, to_tensor, Wq, bq, Wk, bk, Wv, bv)
    res = bass_utils.run_bass_kernel_spmd(nc, in_maps, core_ids=list(range(8)))
    kernel.last_results = res

    output = np.empty((B, S, HID), dtype=np.float32)
    for core in range(8):
        b, g = divmod(core, G)
        j0 = JW * g
        output[b, :, j0:j0 + JW] = res.results[core]["out"].T
    return output


if __name__ == "__main__":
    rng = np.random.default_rng(0)
    ins = {
        "from_tensor": rng.standard_normal((B, S, HID), dtype=np.float32),
        "to_tensor": rng.standard_normal((B, S, HID), dtype=np.float32),
        "attention_mask": np.ones((B, S, S), dtype=np.int32),
        "Wq": rng.standard_normal((HID, HID), dtype=np.float32) * 0.02,
        "bq": rng.standard_normal((HID,), dtype=np.float32) * 0.01,
        "Wk": rng.standard_normal((HID, HID), dtype=np.float32) * 0.02,
        "bk": rng.standard_normal((HID,), dtype=np.float32) * 0.01,
        "Wv": rng.standard_normal((HID, HID), dtype=np.float32) * 0.02,
        "bv": rng.standard_normal((HID,), dtype=np.float32) * 0.01,
    }
    got = kernel(**ins)
    want = _numpy_fallback(**ins)
    err = np.abs(got - want).max() / np.abs(want).max()
    print("self-test rel err:", err)
